# revision 34
# baseline (speedup 1.0000x reference)
"""CondConv2d Trainium2 kernel.

Per-sample expert-combined 3x3 conv (B=16, 256->256 ch, 64x64, fp32),
data-parallel over batch on 8 NeuronCores (2 samples/core).

Device algorithm per core:
  1. Expert combine W_b = sum_e r_be * bank_e, with the bank pre-transposed
     host-side to [e, co_half, kh*kw, ci, co128] so combined weights land
     directly in matmul-ready [ci, co] layout. The combine is split by
     output-channel half to pipeline with the conv:
       - co-half 0 on the PE (diag(r_be).T @ bank_e accumulated in PSUM),
         hidden inside the initial bank DMA window;
       - co-half 1 on the (otherwise idle) DVE via FMA chains, hidden under
         the co-half-0 conv.
  2. Implicit-GEMM conv per co-half: out[co128, pix] accumulated over 18
     matmuls (2 ci-tiles x 9 kernel positions); rhs = shifted windows of a
     zero-padded input image in SBUF; N = 8 rows x 64 cols = 512 per matmul.
All matmuls use fp32r (FP22 multiply, full PE rate at N>=256).
"""

import os

import numpy as np

import concourse.bass as bass
import concourse.tile as tile
from concourse import bacc, mybir
from concourse.bass_utils import run_bass_kernel_spmd

B, C_IN, C_OUT, H, W = 16, 256, 256, 64, 64
KH = KW = 3
KK = KH * KW
E = 8
N_CORES = 8
BPC = B // N_CORES  # samples per core

HP, WP = H + 2, W + 2  # zero-padded image dims
CI_T = C_IN // 128
CO_T = C_OUT // 128
KCOH = KK * 128  # per-co-half free dim of combined weights: (khkw, co128)
CCH = 3 * 128  # PE-combine chunk: 3 kernel positions x 128 co = 384
PIX_ROWS = 8  # image rows per conv matmul -> N = 8*64 = 512

F32 = mybir.dt.float32
F32R = mybir.dt.float32r
U32 = mybir.dt.uint32
Alu = mybir.AluOpType

LAST_RESULTS = None  # stashed BassKernelResults for test harness introspection
_NC_CACHE = []


def _build():
    nc = bacc.Bacc("TRN2", target_bir_lowering=False, debug=False, enable_asserts=False)
    x_d = nc.dram_tensor("x", [BPC, C_IN, H, W], F32R, kind="ExternalInput")
    bank_d = nc.dram_tensor("bank", [E, CO_T, KK, C_IN, 128], F32R, kind="ExternalInput")
    # sid[p, b*E+e, c] = routing[b, e] if p == c else 0  (built on host)
    sid_d = nc.dram_tensor("sid", [128, BPC * E * 128], F32R, kind="ExternalInput")
    rout_d = nc.dram_tensor("rout", [128, BPC * E], F32, kind="ExternalInput")
    out_d = nc.dram_tensor("out", [BPC, C_OUT, H, W], F32, kind="ExternalOutput")

    with tile.TileContext(nc) as tc:
        with (
            tc.tile_pool(name="const", bufs=1) as constp,
            tc.tile_pool(name="xpad", bufs=1) as xpadp,
            tc.tile_pool(name="wcomb", bufs=1) as wcombp,
            tc.tile_pool(name="bank0", bufs=6) as bank0p,
            tc.tile_pool(name="bank1", bufs=4) as bank1p,
            tc.tile_pool(name="xstg", bufs=3) as xstgp,
            tc.tile_pool(name="outs", bufs=4) as outsp,
            tc.tile_pool(name="psum", bufs=8, space="PSUM") as psump,
        ):
            # sid is e-major [e, b]: expert 0's identities land first so the
            # first combine matmuls gate on 128KB, not the full megabyte.
            # The rest of sid (and rout, needed only by the DVE combine) are
            # issued after the first bank tile inside the combine loop.
            sid = constp.tile([128, BPC * E * 128], F32R, tag="sid")
            nc.sync.dma_start(sid[:, 0 : BPC * 128], sid_d[:, 0 : BPC * 128])
            rout = constp.tile([128, BPC * E], F32, tag="rout")

            # Zero-padded input images ([H+2, W+2] per ci-partition). Only
            # the halo is memset; the interior arrives via contiguous
            # full-rate DMA into a small staging tile + GpSimd scatter-copy
            # (strided SBUF writes are cheap on-chip, expensive for DMA).
            xpad = {}
            for b in range(BPC):
                for ct in range(CI_T):
                    t = xpadp.tile([128, HP * WP], F32R, tag=f"xpad{b}{ct}", name=f"xpad{b}{ct}")
                    u = t.bitcast(U32)
                    nc.gpsimd.memset(u[:, 0:WP], 0)  # top pad row
                    nc.gpsimd.memset(u[:, (HP - 1) * WP :], 0)  # bottom pad row
                    # side pads: pairs (row r col W+1, row r+1 col 0)
                    nc.gpsimd.memset(
                        u[:, WP - 1 : WP - 1 + 65 * WP].rearrange("p (h w) -> p h w", h=65)[:, :, 0:2],
                        0,
                    )
                    xpad[(b, ct)] = t

            # Separate combined-weight tiles per co-half: half 0 is written by
            # the ScalarE (PSUM evictions), half 1 by the DVE FMA chains.
            # Separate tiles keep the DVE stream free of false WAW deps on
            # the ScalarE evictions.
            wcomb = {}
            for b in range(BPC):
                for ct in range(CI_T):
                    for cot in range(CO_T):
                        wcomb[(b, ct, cot)] = wcombp.tile(
                            [128, KCOH], F32R, tag=f"wc{b}{ct}{cot}", name=f"wc{b}{ct}{cot}"
                        )

            # Banded input-image loads: 6 row-bands per (sample, ci-tile) so
            # they interleave with the bank streams and unlock early conv
            # pix-groups via subtile deps.
            ROW_BANDS = [(0, 22), (22, 43), (43, 64)]

            def load_x_band(b, band):
                r0, r1 = ROW_BANDS[band]
                nrows = r1 - r0
                for ct in range(CI_T):
                    stg = xstgp.tile([128, max(r1 - r0 for r0, r1 in ROW_BANDS) * W],
                                     F32R, tag="xstg", name="xstg")
                    nc.sync.dma_start(
                        stg[:, 0 : nrows * W],
                        x_d[b, ct * 128 : (ct + 1) * 128, r0:r1, :].rearrange(
                            "ci h w -> ci (h w)"
                        ),
                    )
                    v = xpad[(b, ct)].rearrange("p (h w) -> p h w", h=HP)
                    nc.gpsimd.tensor_copy(
                        v[:, 1 + r0 : 1 + r1, 1 : W + 1],
                        stg[:, 0 : nrows * W].rearrange("p (h w) -> p h w", h=nrows),
                    )

            # ---- co-half 0 combine on the PE (streams behind bank DMA),
            #      x(b=0) bands interleaved into the same DMA window.
            # e-major: one [128, 9*128] DMA per (ct, e); 6 PSUM chunk-tiles
            # per ct accumulate across the 8 experts.
            it = 0
            for ct in range(CI_T):
                pcs = {
                    (c, b): psump.tile([128, PIX_ROWS * W], F32, tag="ps", name="ps")
                    for c in range(KCOH // CCH)
                    for b in range(BPC)
                }
                for e in range(E):
                    bk = bank0p.tile([128, KCOH], F32R, tag="bank0", name="bank0")
                    nc.sync.dma_start(
                        bk[:].rearrange("p (k co) -> p k co", k=KK),
                        bank_d[e, 0, :, ct * 128 : (ct + 1) * 128, :]
                        .rearrange("k ci co -> ci k co"),
                    )
                    if ct == 0 and e == 0:
                        nc.sync.dma_start(sid[:, BPC * 128 :], sid_d[:, BPC * 128 :])
                        nc.sync.dma_start(rout[:], rout_d[:])
                    for c in range(KCOH // CCH):
                        for b in range(BPC):
                            nc.tensor.matmul(
                                pcs[(c, b)][:, 0:CCH],
                                sid[:, (e * BPC + b) * 128 : (e * BPC + b + 1) * 128],
                                bk[:, c * CCH : (c + 1) * CCH],
                                start=(e == 0),
                                stop=(e == E - 1),
                            )
                    it += 1
                for c in range(KCOH // CCH):
                    for b in range(BPC):
                        nc.scalar.copy(
                            wcomb[(b, ct, 0)][:, c * CCH : (c + 1) * CCH],
                            pcs[(c, b)][:, 0:CCH],
                        )

            # ---- co-half 1 combine on the DVE (hidden under co-half 0 conv);
            #      x(b=1) bands interleaved mid-stream ----
            def combine_dve(ct):
                for e in range(E):
                    bk = bank1p.tile([128, KCOH], F32R, tag="bank1", name="bank1")
                    nc.sync.dma_start(
                        bk[:].rearrange("p (k co) -> p k co", k=KK),
                        bank_d[e, 1, :, ct * 128 : (ct + 1) * 128, :]
                        .rearrange("k ci co -> ci k co"),
                    )
                    for b in range(BPC):
                        wslice = wcomb[(b, ct, 1)][:]
                        rsc = rout[:, b * E + e : b * E + e + 1]
                        if e == 0:
                            nc.vector.tensor_scalar_mul(wslice, bk[:], rsc)
                        else:
                            nc.vector.scalar_tensor_tensor(
                                wslice, bk[:], rsc, wslice, Alu.mult, Alu.add
                            )

            load_x_band(0, 0)
            load_x_band(0, 1)
            load_x_band(0, 2)
            combine_dve(0)
            load_x_band(1, 0)
            load_x_band(1, 1)
            load_x_band(1, 2)
            combine_dve(1)

            # ---- conv as implicit GEMM, co-half major ----
            for cot in range(CO_T):
                for b in range(BPC):
                    for p in range(H // PIX_ROWS):
                        h0 = p * PIX_ROWS
                        pc = psump.tile([128, PIX_ROWS * W], F32, tag="ps", name="ps")
                        first = True
                        for ct in range(CI_T):
                            xv = xpad[(b, ct)].rearrange("p (h w) -> p h w", h=HP)
                            for kh in range(KH):
                                for kw in range(KW):
                                    kk = kh * KW + kw
                                    lhsT = wcomb[(b, ct, cot)][
                                        :, kk * 128 : (kk + 1) * 128
                                    ]
                                    rhs = xv[:, h0 + kh : h0 + kh + PIX_ROWS, kw : kw + W]
                                    last = ct == CI_T - 1 and kk == KK - 1
                                    nc.tensor.matmul(pc[:], lhsT, rhs, start=first, stop=last)
                                    first = False
                        ot = outsp.tile([128, PIX_ROWS * W], F32, tag="outs", name="outs")
                        nc.scalar.copy(ot[:], pc[:])
                        nc.sync.dma_start(
                            out_d[b, cot * 128 : (cot + 1) * 128, h0 : h0 + PIX_ROWS, :],
                            ot.rearrange("p (h w) -> p h w", h=PIX_ROWS),
                        )
    nc.compile()
    return nc


def kernel(x, routing_weights, expert_weight):
    global LAST_RESULTS
    x = np.ascontiguousarray(np.asarray(x, dtype=np.float32))
    r = np.asarray(routing_weights, dtype=np.float32)
    bank = np.asarray(expert_weight, dtype=np.float32)

    # Host relayout: [e, co*ci*kh*kw] -> [e, co_half, kh*kw, ci, co128] so
    # combined weights come out of the device combine in matmul-ready
    # [ci, co] tiles, co-half major.
    bank_t = np.ascontiguousarray(
        bank.reshape(E, CO_T, 128, C_IN, KK).transpose(0, 1, 4, 3, 2)
    )

    if not _NC_CACHE:
        _NC_CACHE.append(_build())
    nc = _NC_CACHE[0]

    in_maps = []
    for c in range(N_CORES):
        rows = r[c * BPC : (c + 1) * BPC].reshape(BPC * E)
        # Scaled identities: diag embedding of the routing values, e-major.
        rows_e_major = r[c * BPC : (c + 1) * BPC].T.reshape(E * BPC)
        sid = np.zeros((128, E * BPC, 128), dtype=np.float32)
        idx = np.arange(128)
        sid[idx, :, idx] = rows_e_major[None, :]
        in_maps.append(
            {
                "x": np.ascontiguousarray(x[c * BPC : (c + 1) * BPC]),
                "bank": bank_t,
                "sid": np.ascontiguousarray(sid.reshape(128, BPC * E * 128)),
                "rout": np.ascontiguousarray(
                    np.broadcast_to(rows[None, :], (128, BPC * E))
                ),
            }
        )

    trace = bool(os.environ.get("KERNEL_TRACE"))
    try:
        res = run_bass_kernel_spmd(
            nc, in_maps, core_ids=list(range(N_CORES)), trace=trace
        )
    except ModuleNotFoundError:
        if not trace:
            raise
        # Tracing unavailable in this environment (no axon NTFF hook).
        res = run_bass_kernel_spmd(
            nc, in_maps, core_ids=list(range(N_CORES)), trace=False
        )
    LAST_RESULTS = res
    return np.concatenate([rr["out"] for rr in res.results], axis=0)
